# revision 1
# baseline (speedup 1.0000x reference)
"""Trainium2 Bass kernel for nn_AttentiveEncoder (embed -> linear -> full self-attention).

Sharding: query rows split across 8 NeuronCores (1024 rows each). Each core:
  phase A: gathers its 1024 embedding rows (dma_gather), computes
           L = E @ W.T + b in fp32r on the tensor engine, stages the bf16
           L-shard to local DRAM (incrementally, so the first collective can
           start early).
  exchange: 4 chunked AllGathers; chunk g gathers rows [256g, 256g+256) of
           every core's shard -> gathered_g [2048, 1024] bf16. Each later
           collective is emitted after the previous chunk's loads so it
           overlaps that chunk's attention (the Tile xbar rule would otherwise
           serialize collectives against DMA-transposes, so the transposed KV
           copy uses an SBUF-source dma_gather with identity indices instead,
           split into 512-index pieces to fit the SWDGE descriptor ring).
  attention: per chunk, the 2048 keys are SBUF-resident in natural layout
           (kv_nat, j%128-partitioned) and transposed layout (kv_t quarters).
           Per (q-group of 256, 128-key j-tile): S_T = KV_T.T @ Q_T on the
           tensor engine, P = exp(S/sqrt(H)) on ACT written bf16, then
           out += P.T @ KV and denominator += P.T @ ones accumulated in PSUM
           across the whole chunk. DVE flushes PSUM->SBUF once per
           (chunk, q-group); the last chunk's flush also does the reciprocal
           normalize + store so the tail overlaps remaining compute.
"""
import numpy as np
from contextlib import ExitStack

import concourse.bass as bass
import concourse.bacc as bacc
import concourse.tile as tile
from concourse.tile import add_dep_helper
from concourse import mybir
from concourse.bass_utils import run_bass_kernel_spmd

F32 = mybir.dt.float32
F32R = mybir.dt.float32r
BF16 = mybir.dt.bfloat16
I16 = mybir.dt.int16

N_CORES = 8
VOCAB = 32000
H = 1024             # hidden
SEQ = 8192           # sequence
NQ = SEQ // N_CORES  # query rows per core (1024)
KT = H // 128        # k-tiles over hidden (8)
CHUNK_ROWS = [256, 256, 256, 256]   # rows per core per collective chunk
CHUNK_OFF = [0, 256, 512, 768]
G = len(CHUNK_ROWS)
QGS = 256            # query rows per q-group
NQG = NQ // QGS      # q-groups per core (4)
IC = QGS // 128      # i-chunks per q-group (2)
HC = H // 512        # h-chunks (2)
SCALE = 1.0 / np.sqrt(np.float32(H))

_cached = None


def _build(sim_single_core=False):
    nc = bacc.Bacc()

    ids16 = nc.dram_tensor("ids16", [128, NQ // 16], I16, kind="ExternalInput")
    emb = nc.dram_tensor("emb", [VOCAB, H], F32, kind="ExternalInput")
    wt = nc.dram_tensor("wt", [H, H], F32, kind="ExternalInput")      # W.T (k-major)
    bias = nc.dram_tensor("bias", [1, H], F32, kind="ExternalInput")
    ident = nc.dram_tensor("ident", [128, 128], F32, kind="ExternalInput")
    seqid = nc.dram_tensor("seqid", [128, 144], I16, kind="ExternalInput")  # 0..2303 wrapped
    out_d = nc.dram_tensor("out", [NQ, H], F32, kind="ExternalOutput")

    with tile.TileContext(nc) as tc, ExitStack() as ctx:
        pers = ctx.enter_context(tc.tile_pool(name="pers", bufs=1))
        dram = ctx.enter_context(tc.tile_pool(name="dram", bufs=1, space="DRAM"))

        q_t = pers.tile([128, KT, NQ], BF16, tag="q_t")           # Q^T [h, i]
        out_acc = pers.tile([128, NQ // 128, H], F32, tag="out_acc")
        den_acc = pers.tile([128, NQ // 128], F32, tag="den_acc")
        ones_bf = pers.tile([128, 1], BF16, tag="ones_bf")
        nc.vector.memset(ones_bf[:], 1.0)
        seq_sb = pers.tile([128, 144], I16, tag="seq_sb")
        nc.sync.dma_start(seq_sb[:], seqid[:])

        l_stage = dram.tile([NQ, H], BF16)                         # local L shard
        gath = [dram.tile([N_CORES * CHUNK_ROWS[g], H], BF16, addr_space="Shared",
                          name=f"gathered{g}") for g in range(G)]

        # ---------------- phase A ----------------
        with tc.tile_pool(name="pa", bufs=1) as pa, \
             tc.tile_pool(name="pa_ps", bufs=2, space="PSUM") as pa_ps, \
             tc.tile_pool(name="pa_ps2", bufs=2, space="PSUM") as pa_ps2:
            ids_sb = pa.tile([128, NQ // 16], I16)
            nc.sync.dma_start(ids_sb[:], ids16[:])
            e_nat = pa.tile([128, NQ // 128, H], F32, tag="e_nat")
            # ascending piece sizes: i-tile 0 lands first so phase A's first
            # matmuls (and therefore AG0) start as early as possible
            for p0, p1 in [(0, 1), (1, 2), (2, 4), (4, 8)]:
                nc.gpsimd.dma_gather(
                    out_ap=e_nat[:, p0:p1, :], in_ap=emb[:],
                    idxs_ap=ids_sb[:, p0 * 8:p1 * 8],
                    num_idxs=(p1 - p0) * 128, num_idxs_reg=(p1 - p0) * 128,
                    elem_size=H,
                )

            id_sb = pa.tile([128, 128], F32, tag="id_sb")
            nc.sync.dma_start(id_sb[:], ident[:])

            w_sb = pa.tile([128, KT, H], F32, tag="w_sb")
            nc.sync.dma_start(w_sb[:], wt.rearrange("(kt p) h -> p kt h", p=128))
            w_r = pa.tile([128, KT, H], F32R, tag="w_r")
            nc.vector.tensor_copy(w_r[:], w_sb[:])

            b_sb = pa.tile([1, H], F32, tag="b_sb")
            nc.sync.dma_start(b_sb[:], bias[:])
            b_r = pa.tile([1, H], F32R, tag="b_r")
            nc.vector.tensor_copy(b_r[:], b_sb[:])
            one_f = pa.tile([1, 128], F32, tag="one_f")
            nc.vector.memset(one_f[:], 1.0)
            one_r = pa.tile([1, 128], F32R, tag="one_r")
            nc.vector.tensor_copy(one_r[:], one_f[:])

            # Per i-tile: E^T via PE transposes, then L = E @ W.T + b (fp32r),
            # written bf16 and staged to DRAM incrementally so AG0 starts early.
            e_t = pa.tile([128, KT, NQ], F32R, tag="e_t")
            l_bf = pa.tile([128, NQ // 128, H], BF16, tag="l_bf")
            l_stage_r = l_stage.rearrange("(a p) h -> p a h", p=128)
            for it in range(NQ // 128):
                for kt in range(KT):
                    tp = pa_ps.tile([128, 128], F32, tag="tp")
                    nc.tensor.transpose(tp[:], e_nat[:, it, kt * 128:(kt + 1) * 128], id_sb[:])
                    nc.vector.tensor_copy(e_t[:, kt, it * 128:(it + 1) * 128], tp[:])
                ps = pa_ps2.tile([128, HC, 512], F32, tag="ps")
                for hc in range(HC):
                    for kt in range(KT):
                        nc.tensor.matmul(
                            ps[:, hc, :],
                            e_t[:, kt, it * 128:(it + 1) * 128],
                            w_r[:, kt, hc * 512:(hc + 1) * 512],
                            start=(kt == 0), stop=False,
                        )
                    nc.tensor.matmul(
                        ps[:, hc, :], one_r[:], b_r[:, hc * 512:(hc + 1) * 512],
                        start=False, stop=True,
                    )
                    nc.scalar.copy(l_bf[:, it, hc * 512:(hc + 1) * 512], ps[:, hc, :])
                # staged via the ACT HWDGE queues so the collectives' queue
                # waits aren't entangled with the SP-queue kv loads
                if it < 2:
                    nc.scalar.dma_start(l_stage_r[:, it:it + 1, :], l_bf[:, it:it + 1, :])
                elif it % 2 == 1:
                    nc.scalar.dma_start(l_stage_r[:, it - 1:it + 1, :], l_bf[:, it - 1:it + 1, :])

        # Q^T from own shard (dma transpose); runs before any collective, so
        # the xbar transpose<->collective serialization cannot stall it.
        for ht in range(KT):
            nc.sync.dma_start_transpose(q_t[:, ht, :], l_stage[:, ht * 128:(ht + 1) * 128])

        # ---------------- chunked collectives + attention ----------------
        if not sim_single_core:
            nc.gpsimd.collective_compute(
                "AllGather", mybir.AluOpType.bypass,
                replica_groups=[list(range(N_CORES))],
                ins=[l_stage[0:CHUNK_ROWS[0], :]], outs=[gath[0].opt()],
            )

        with tc.tile_pool(name="kv", bufs=2) as kvp, \
             tc.tile_pool(name="pt", bufs=4) as ptp, \
             tc.tile_pool(name="st_ps", bufs=2, space="PSUM") as st_ps, \
             tc.tile_pool(name="out_ps", bufs=1, space="PSUM") as out_ps, \
             tc.tile_pool(name="den_ps", bufs=1, space="PSUM") as den_ps, \
             tc.tile_pool(name="fin", bufs=2) as fin:
            for g in range(G):
                src = l_stage if sim_single_core else gath[g]
                nblk = N_CORES

                # loads: one natural-layout copy + transposed gathers
                CJ = nblk * CHUNK_ROWS[g]  # keys per chunk
                kv_nat = kvp.tile([128, CJ // 128, H], BF16, tag="kv_nat",
                                  name=f"kv_nat{g}")
                ld_nat = nc.sync.dma_start(
                    kv_nat[:], src[0:CJ, :].rearrange("(a p) h -> p a h", p=128))
                # transposed copy via SBUF-source dma_gather (identity idxs):
                # rank=j//128, tok=j%128 reads kv_nat[j%128, j//128, :], i.e. the
                # natural tile, and transpose-writes [h, j] — no xbar DMA class,
                # so it does not serialize against the collectives.
                # split into 512-idx gathers: the SWDGE ring holds only
                # dynamic_dma_scratch_size/16 = 1024 descriptors, and a gather
                # needs one per index. One tile per quarter (gather output
                # must be contiguous).
                kv_t = []
                j_done = 0
                q4 = 0
                while j_done < CJ:
                    n = min(512, CJ - j_done)
                    ktq = kvp.tile([128, KT, n], BF16, tag=f"kv_t{q4}",
                                   name=f"kv_t{g}_{q4}")
                    nc.gpsimd.dma_gather(
                        out_ap=ktq[:], in_ap=kv_nat[:],
                        idxs_ap=seq_sb[:, j_done // 16:(j_done + n) // 16],
                        num_idxs=n, num_idxs_reg=n, elem_size=H, transpose=True,
                        sbuf_tokens_per_rank=128, sbuf_free_dim_per_rank=2 * H,
                    )
                    kv_t.append(ktq)
                    j_done += n
                    q4 += 1
                if not sim_single_core and g + 1 < G:
                    r0, r1 = CHUNK_OFF[g + 1], CHUNK_OFF[g + 1] + CHUNK_ROWS[g + 1]
                    ag = nc.gpsimd.collective_compute(
                        "AllGather", mybir.AluOpType.bypass,
                        replica_groups=[list(range(N_CORES))],
                        ins=[l_stage[r0:r1, :]],
                        outs=[gath[g + 1].opt()],
                    )
                    add_dep_helper(ag.ins, ld_nat.ins, sync=False,
                                   reason="AG after this chunk's loads")

                for qg in range(NQG):
                    ops = out_ps.tile([128, 2 * HC, 512], F32, tag="ops")
                    dps = [den_ps.tile([128, 1], F32, tag=f"dps{ic}",
                                       name=f"dps{g}_{qg}_{ic}") for ic in range(IC)]
                    NJT = CJ // 128  # j-tiles per chunk
                    for jt in range(NJT):
                        st = st_ps.tile([128, QGS], F32, tag="st")
                        for ht in range(KT):
                            nc.tensor.matmul(
                                st[:],
                                kv_t[jt // 4][:, ht, (jt % 4) * 128:(jt % 4 + 1) * 128],
                                q_t[:, ht, qg * QGS:(qg + 1) * QGS],
                                start=(ht == 0), stop=(ht == KT - 1),
                            )  # jt//4 valid: 512-idx gather pieces = 4 j-tiles each
                        p_t = ptp.tile([128, QGS], BF16, tag="p_t")
                        nc.scalar.activation(p_t[:], st[:],
                                             mybir.ActivationFunctionType.Exp,
                                             scale=float(SCALE))
                        first, last = (jt == 0), (jt == NJT - 1)
                        for ic in range(IC):
                            lhs = p_t[:, ic * 128:(ic + 1) * 128]
                            for hc in range(HC):
                                nc.tensor.matmul(
                                    ops[:, ic * HC + hc, :],
                                    lhs, kv_nat[:, jt, hc * 512:(hc + 1) * 512],
                                    start=first, stop=last,
                                )
                            nc.tensor.matmul(
                                dps[ic][:], lhs, ones_bf[:],
                                start=first, stop=last,
                            )
                    # flush psum accumulators into SBUF accumulators;
                    # on the last chunk, normalize + store this q-group
                    # immediately so the tail overlaps remaining compute.
                    out_r = out_d.rearrange("(a p) h -> p a h", p=128)
                    for ic in range(IC):
                        gi = qg * IC + ic
                        acc = out_acc[:, gi, :]
                        pslice = ops[:, ic * HC:(ic + 1) * HC, :]
                        if g == 0:
                            nc.vector.tensor_copy(acc, pslice.opt())
                            nc.vector.tensor_copy(den_acc[:, gi:gi + 1], dps[ic][:])
                        else:
                            nc.vector.tensor_add(acc, acc, pslice.opt())
                            nc.vector.tensor_add(den_acc[:, gi:gi + 1],
                                                 den_acc[:, gi:gi + 1], dps[ic][:])
                        if g == G - 1:
                            recip = pers.tile([128, 1], F32, tag=f"recip{gi}",
                                              name=f"recip{gi}")
                            nc.vector.reciprocal(recip[:], den_acc[:, gi:gi + 1])
                            o = fin.tile([128, H], F32, tag="o")
                            nc.vector.tensor_scalar_mul(o[:], acc, recip[:])
                            nc.sync.dma_start(out_r[:, gi, :], o[:])

    nc.compile()
    return nc


def _get_nc():
    global _cached
    if _cached is None:
        _cached = _build()
    return _cached


last_results = None
_last_in_maps = None


def kernel(input, emb_table, W, b):
    global last_results
    nc = _get_nc()

    ids = np.asarray(input).astype(np.int64)
    emb_np = np.ascontiguousarray(np.asarray(emb_table, dtype=np.float32))
    wt_np = np.ascontiguousarray(np.asarray(W, dtype=np.float32).T)
    b_np = np.ascontiguousarray(np.asarray(b, dtype=np.float32).reshape(1, H))
    ident_np = np.eye(128, dtype=np.float32)
    sq = np.arange(2304, dtype=np.int16)
    seqid_np = np.tile(sq.reshape(144, 16).T, (8, 1)).copy()

    in_maps = []
    for c in range(N_CORES):
        shard = ids[c * NQ:(c + 1) * NQ].astype(np.int16)
        # idx i lives at [i % 16, i // 16], replicated across the 8 partition groups
        wrapped = np.tile(shard.reshape(NQ // 16, 16).T, (8, 1)).copy()
        in_maps.append({
            "ids16": wrapped, "emb": emb_np, "wt": wt_np,
            "bias": b_np, "ident": ident_np, "seqid": seqid_np,
        })

    global _last_in_maps
    _last_in_maps = in_maps
    res = run_bass_kernel_spmd(nc, in_maps, list(range(N_CORES)))
    last_results = res
    return np.concatenate([res.results[c]["out"] for c in range(N_CORES)], axis=0)

